# revision 4
# baseline (speedup 1.0000x reference)
"""NetVLAD Trainium2 kernel (8-core SPMD, data-parallel over batch).

Per-core pipeline (8 samples each):
  mm1:  s^T[k,hw] = W^T @ x^T          (W stationary f16, x^T moving f16)
  ACT:  e^T = exp(s^T + b)             (bias per-partition k)
  PE :  transpose e^T -> e (natural)   (8x [64,128] transposes)
  DVE:  S = sum_k e ; a = e / S        (softmax over k)
  mm2:  v^T[k,d] = a^T @ x  (+ diag(a_sum) @ C^T folded into same PSUM accum)
  ACT/DVE: intra-norm over d, global L2 norm (exact, eps=1e-12)
  out:  v_hat^T [64,512] f32 per sample; host transposes to [512*64] layout
"""

import os
import sys

import numpy as np

for _p in ("/opt/trn_rl_repo", "/root/.axon_site/_ro/trn_rl_repo"):
    if os.path.isdir(_p) and _p not in sys.path:
        sys.path.append(_p)

from contextlib import ExitStack

from concourse import bacc, bass, mybir, tile
from concourse.bass_utils import run_bass_kernel_spmd

N_CORES = 8
NSAMP = 8        # samples per core
HW = 1024        # H*W
D = 512
K = 64
DC = D // 128    # d-chunks of 128
HC = HW // 128   # hw-chunks of 128
F16 = mybir.dt.float16
F32 = mybir.dt.float32
EPS = 1e-12

LAST_EXEC_NS = None
LAST_RESULTS = None

_CACHE = {}


def _build_program():
    nc = bacc.Bacc("TRN2", target_bir_lowering=False, debug=False)

    xT_d = nc.dram_tensor("xT", [NSAMP, D, HW], F16, kind="ExternalInput").ap()
    xn_d = nc.dram_tensor("xn", [NSAMP, HW, D], F16, kind="ExternalInput").ap()
    W_d = nc.dram_tensor("Wt", [DC, 128, K], F16, kind="ExternalInput").ap()
    b_d = nc.dram_tensor("bcol", [K, 1], F32, kind="ExternalInput").ap()
    CT_d = nc.dram_tensor("CT", [K, D], F16, kind="ExternalInput").ap()
    eye_d = nc.dram_tensor("eye64", [K, K], F16, kind="ExternalInput").ap()
    ones128_d = nc.dram_tensor("ones128", [128, 1], F16, kind="ExternalInput").ap()
    ones64_d = nc.dram_tensor("ones64", [K, 1], F32, kind="ExternalInput").ap()
    onesr_d = nc.dram_tensor("onesr", [1, K], F32, kind="ExternalInput").ap()
    eps_d = nc.dram_tensor("epscol", [K, 1], F32, kind="ExternalInput").ap()
    out_d = nc.dram_tensor("out", [NSAMP, K, D], F32, kind="ExternalOutput").ap()

    with tile.TileContext(nc) as tc, ExitStack() as ctx:
        const = ctx.enter_context(tc.tile_pool(name="const", bufs=1))
        xT_pool = ctx.enter_context(tc.tile_pool(name="xTp", bufs=3))
        xn_pool = ctx.enter_context(tc.tile_pool(name="xnp", bufs=3))
        eT_pool = ctx.enter_context(tc.tile_pool(name="eTp", bufs=2))
        a_pool = ctx.enter_context(tc.tile_pool(name="ap", bufs=2))
        sm_pool = ctx.enter_context(tc.tile_pool(name="smp", bufs=2))
        scr_pool = ctx.enter_context(tc.tile_pool(name="scrp", bufs=2))
        vo_pool = ctx.enter_context(tc.tile_pool(name="vop", bufs=2))

        ps_sT = ctx.enter_context(tc.tile_pool(name="ps_sT", bufs=2, space="PSUM"))
        ps_e = ctx.enter_context(tc.tile_pool(name="ps_e", bufs=1, space="PSUM"))
        ps_v = ctx.enter_context(tc.tile_pool(name="ps_v", bufs=2, space="PSUM"))
        ps_t = ctx.enter_context(tc.tile_pool(name="ps_t", bufs=1, space="PSUM"))

        # constants
        W_sb = const.tile([128, DC, K], F16)
        nc.sync.dma_start(W_sb[:], W_d[:].rearrange("dc p k -> p dc k"))
        b_sb = const.tile([K, 1], F32)
        nc.sync.dma_start(b_sb[:], b_d[:])
        CT_sb = const.tile([K, D], F16)
        nc.sync.dma_start(CT_sb[:], CT_d[:])
        eye_sb = const.tile([K, K], F16)
        nc.sync.dma_start(eye_sb[:], eye_d[:])
        ones128_sb = const.tile([128, 1], F16)
        nc.sync.dma_start(ones128_sb[:], ones128_d[:])
        ones64_sb = const.tile([K, 1], F32)
        nc.sync.dma_start(ones64_sb[:], ones64_d[:])
        onesr_sb = const.tile([1, K], F32)
        nc.sync.dma_start(onesr_sb[:], onesr_d[:])
        eps_sb = const.tile([K, 1], F32)
        nc.sync.dma_start(eps_sb[:], eps_d[:])

        for n in range(NSAMP):
            # ---- loads ----
            xT_t = xT_pool.tile([128, DC, HW], F16, name=f"xT_{n}", tag="xT")
            nc.sync.dma_start(xT_t[:], xT_d[n].rearrange("(dc p) f -> p dc f", dc=DC))
            xn_t = xn_pool.tile([128, HC, D], F16, name=f"xn_{n}", tag="xn")
            nc.sync.dma_start(xn_t[:], xn_d[n].rearrange("(c p) d -> p c d", c=HC))

            # ---- mm1: s^T = W^T @ x^T  [64, 1024] ----
            sT_ps = ps_sT.tile([K, HW], F32, name=f"sT_{n}", tag="sT")
            for h in range(2):
                for dc in range(DC):
                    nc.tensor.matmul(
                        sT_ps[:, h * 512:(h + 1) * 512],
                        lhsT=W_sb[:, dc, :],
                        rhs=xT_t[:, dc, h * 512:(h + 1) * 512],
                        start=(dc == 0),
                        stop=(dc == DC - 1),
                    )

            # ---- exp(s^T + b) -> e^T f16 ----
            eT_sb = eT_pool.tile([K, HW], F16, name=f"eT_{n}", tag="eT")
            nc.scalar.activation(
                eT_sb[:], sT_ps[:],
                mybir.ActivationFunctionType.Exp,
                bias=b_sb[:], scale=1.0,
            )

            # ---- transpose e^T -> e natural [128, 8, 64] (PSUM f16) ----
            e_ps = ps_e.tile([128, HC, K], F16, name=f"e_{n}", tag="e")
            for c in range(HC):
                nc.tensor.transpose(
                    e_ps[:, c, :], eT_sb[:, c * 128:(c + 1) * 128], eye_sb[:]
                )

            # ---- softmax denominators and normalize ----
            S_sb = sm_pool.tile([128, HC], F32, name=f"S_{n}", tag="S")
            nc.vector.reduce_sum(S_sb[:], e_ps[:], axis=mybir.AxisListType.X)
            r_sb = sm_pool.tile([128, HC], F32, name=f"r_{n}", tag="r")
            nc.vector.reciprocal(r_sb[:], S_sb[:])
            a_sb = a_pool.tile([128, HC, K], F16, name=f"a_{n}", tag="a")
            for c in range(HC):
                nc.vector.tensor_scalar(
                    a_sb[:, c, :], e_ps[:, c, :],
                    r_sb[:, c:c + 1], None,
                    op0=mybir.AluOpType.mult,
                )

            # ---- mm2: v^T = a^T @ x (+ a_sum, + diag(a_sum) @ C^T) ----
            v_ps = ps_v.tile([K, D], F32, name=f"v_{n}", tag="v")
            tiny_ps = ps_t.tile([K, 4], F32, name=f"tiny_{n}", tag="tiny")
            for c in range(HC):
                nc.tensor.matmul(
                    v_ps[:],
                    lhsT=a_sb[:, c, :],
                    rhs=xn_t[:, c, :],
                    start=(c == 0),
                    stop=False,
                    skip_group_check=True,
                )
                nc.tensor.matmul(
                    tiny_ps[:, 0:1],
                    lhsT=a_sb[:, c, :],
                    rhs=ones128_sb[:],
                    start=(c == 0),
                    stop=(c == HC - 1),
                    skip_group_check=True,
                )
            diag_sb = sm_pool.tile([K, K], F16, name=f"diag_{n}", tag="diag")
            nc.vector.tensor_scalar(
                diag_sb[:], eye_sb[:], tiny_ps[:, 0:1], None,
                op0=mybir.AluOpType.mult,
            )
            nc.tensor.matmul(
                v_ps[:], lhsT=diag_sb[:], rhs=CT_sb[:],
                start=False, stop=True, skip_group_check=True,
            )

            # ---- intra-norm + global norm ----
            sq_sb = scr_pool.tile([K, D], F16, name=f"sq_{n}", tag="sq")
            ssq_sb = sm_pool.tile([K, 1], F32, name=f"ssq_{n}", tag="ssq")
            nc.scalar.activation(
                sq_sb[:], v_ps[:],
                mybir.ActivationFunctionType.Square,
                accum_out=ssq_sb[:],
            )
            sqr_sb = sm_pool.tile([K, 1], F32, name=f"sqr_{n}", tag="sqr")
            nc.scalar.activation(
                sqr_sb[:], ssq_sb[:],
                mybir.ActivationFunctionType.Sqrt,
                bias=eps_sb[:],
            )
            u_sb = sm_pool.tile([K, 1], F32, name=f"u_{n}", tag="u")
            nc.vector.reciprocal(u_sb[:], sqr_sb[:])
            # t = ssq * u * u  (= ssq / (ssq + eps)); total = sum_k t
            t1_sb = sm_pool.tile([K, 1], F32, name=f"t1_{n}", tag="t1")
            nc.vector.tensor_mul(t1_sb[:], ssq_sb[:], u_sb[:])
            t_sb = sm_pool.tile([K, 1], F32, name=f"t_{n}", tag="t")
            nc.vector.tensor_mul(t_sb[:], t1_sb[:], u_sb[:])
            nc.tensor.matmul(
                tiny_ps[0:1, 1:2], lhsT=t_sb[:], rhs=ones64_sb[:],
                start=True, stop=True, skip_group_check=True,
            )
            rt_sb = sm_pool.tile([1, 1], F32, name=f"rt_{n}", tag="rt")
            nc.scalar.activation(
                rt_sb[:], tiny_ps[0:1, 1:2],
                mybir.ActivationFunctionType.Sqrt,
                bias=eps_sb[0:1, :],
            )
            rti_sb = sm_pool.tile([1, 1], F32, name=f"rti_{n}", tag="rti")
            nc.vector.reciprocal(rti_sb[:], rt_sb[:])
            nc.tensor.matmul(
                tiny_ps[:, 2:3], lhsT=onesr_sb[:], rhs=rti_sb[:],
                start=True, stop=True, skip_group_check=True,
            )
            alpha_sb = sm_pool.tile([K, 1], F32, name=f"alpha_{n}", tag="alpha")
            nc.vector.tensor_mul(alpha_sb[:], u_sb[:], tiny_ps[:, 2:3])

            # ---- scale + store ----
            vo_sb = vo_pool.tile([K, D], F32, name=f"vo_{n}", tag="vo")
            nc.scalar.activation(
                vo_sb[:], v_ps[:],
                mybir.ActivationFunctionType.Copy,
                scale=alpha_sb[:],
            )
            nc.sync.dma_start(out_d[n], vo_sb[:])

    nc.compile()
    return nc


def _get_program():
    if "nc" not in _CACHE:
        _CACHE["nc"] = _build_program()
    return _CACHE["nc"]


def kernel(x, W_assign, b_assign, C):
    global LAST_EXEC_NS, LAST_RESULTS

    x = np.asarray(x, dtype=np.float32).reshape(64, HW, D)
    W_assign = np.asarray(W_assign, dtype=np.float32)
    b_assign = np.asarray(b_assign, dtype=np.float32)
    C = np.asarray(C, dtype=np.float32)

    W16 = W_assign.astype(np.float16).reshape(DC, 128, K)
    bcol = b_assign.reshape(K, 1)
    CT16 = np.ascontiguousarray(C.T).astype(np.float16)
    eye16 = np.eye(K, dtype=np.float16)
    ones128 = np.ones((128, 1), dtype=np.float16)
    ones64 = np.ones((K, 1), dtype=np.float32)
    onesr = np.ones((1, K), dtype=np.float32)
    epscol = np.full((K, 1), EPS, dtype=np.float32)

    in_maps = []
    for c in range(N_CORES):
        xs = x[c * NSAMP:(c + 1) * NSAMP]
        xn16 = xs.astype(np.float16)
        xT16 = np.ascontiguousarray(xs.transpose(0, 2, 1)).astype(np.float16)
        in_maps.append({
            "xT": xT16, "xn": xn16, "Wt": W16, "bcol": bcol, "CT": CT16,
            "eye64": eye16, "ones128": ones128, "ones64": ones64, "onesr": onesr,
            "epscol": epscol,
        })

    nc = _get_program()
    trace = bool(int(os.environ.get("KERNEL_TRACE", "0")))
    res = run_bass_kernel_spmd(
        nc, in_maps, core_ids=list(range(N_CORES)), trace=trace,
    )
    LAST_RESULTS = res
    LAST_EXEC_NS = res.exec_time_ns

    out = np.empty((64, D * K), dtype=np.float32)
    for c in range(N_CORES):
        vT = res.results[c]["out"]                    # [NSAMP, K, D]
        out[c * NSAMP:(c + 1) * NSAMP] = (
            vT.transpose(0, 2, 1).reshape(NSAMP, D * K)
        )
    return out
